# revision 22
# baseline (speedup 1.0000x reference)
"""MaxMarginLoss Trainium2 kernel (8 NeuronCores, vocab-sharded), v2.

Math (reference):
    out_norm = l2norm(preds^T over D)            [B,S,D]
    voc_norm = l2norm(emb over D)                [V,D]
    tgt      = emb[target]                       [B,S,D]
    d        = out_norm@voc_norm.T - tgt@voc_norm.T
    jmax     = argmax_v d
    loss     = mean_masked(relu(g + cos[jmax] - cos[target]))

Key identity: d = (out_norm - tgt) @ voc_norm.T  -> ONE matmul.  Per-row
positive scaling keeps the argmax, so each device computes
    slab[s,v] = (preds[s] - n_s*tgt[s]) . voc_norm[v]    ( = n_s * d[s,v] )
in fp8(e4m3) with DoubleRow perf mode (double-pumped PE, 2 k-planes per
instruction; ~2x bf16 at the power-throttled ~1.2GHz PE clock).  Each
[128,1024] PSUM span A is copied to SBUF bf16 by the scalar engine while
span B streams; the DVE then emits pairmax = max(B, bf16(A)) — a 2:1
reduction of the slab with the copy cost folded in (only one PSUM operand
per instruction is legal).  The 16 pairmax tiles are DMA'd out; the host
takes the global argmax over the 2:1-reduced scores, disambiguates the two
pair candidates {v, v+1024} with exact fp32 dots, and finishes the loss
exactly (cos at the winner, cos at target, masked mean).  fp8 noise only
affects which near-tie index wins the argmax; its cosine is then computed
exactly, so the loss error stays ~2e-4.
"""

import os
import sys

import numpy as np

for _p in ("/opt/trn_rl_repo", "/root/.axon_site/_ro/trn_rl_repo"):
    if os.path.isdir(_p) and _p not in sys.path:
        sys.path.insert(0, _p)

import concourse.bass as bass
import concourse.bacc as bacc_mod
import concourse.mybir as mybir
from concourse.tile import TileContext

P = 128
B, S, D, V = 4, 512, 512, 32000
BS = B * S                  # 2048 rows
NCORES = 8
VS = V // NCORES            # 4000 vocab rows per core
VSP = 4096                  # padded vocab per core (zeros beyond VS never win)
NT = BS // P                # 16 row tiles
NH = 2                      # vocab halves per tile ([P, 2048] PSUM each)
GAMMA = 0.5
VSCALE = 16.0               # scale on voc_norm so fp8 entries sit in normal range

F32 = mybir.dt.float32
BF16 = mybir.dt.bfloat16
F8 = mybir.dt.float8e4

_CACHED = {}


def _mm_noload(nc, out, lhsT, rhs, start, stop, perf_mode):
    """InstMatmult with ldweights=False: stream `rhs` against the weights
    already resident in the PE array (loaded by an explicit
    nc.tensor.ldweights).  Mirrors bass's matmul() lowering for our fixed
    128-partition shapes."""
    te = nc.tensor
    keep_dims = {0, 1}  # DoubleRow: dim1 is the k-plane dim
    ifmap_ap = te.lower_ap(rhs.opt(keep_dims), opt=False)
    weights_ap = te.lower_ap(
        lhsT.opt(keep_dims), opt=False, for_matmul_weights=True)
    out_ap = te.lower_ap(out)
    return te.add_instruction(
        mybir.InstMatmult(
            name=te.bass.get_next_instruction_name(),
            replication_resolution=0,
            replication_shift_amnt=0,
            replication_num_rows=0,
            start_tensor_calc=start,
            stop_tensor_calc=stop,
            ins=[ifmap_ap, weights_ap],
            outs=[out_ap],
            perf_mode=perf_mode,
            is_transpose=None,
            ifmap_quant_offset=None,
            weights_quant_offset=None,
            bass_skip_group_check=True,
            tile_position=(0, 0),
            tile_size=(128, 128),
            ldweights=False,
        )
    )


def build_nc():
    nc = bacc_mod.Bacc()
    DR = mybir.MatmulPerfMode.DoubleRow

    # DoubleRow-packed operands: row r = k2*128 + p holds contraction index
    # k2*256 + i*128 + p in plane i (i stored along the free dim).
    eT8 = nc.declare_dram_parameter("eT8", [2 * P, 2 * BS], F8, isOutput=False)
    vocT8 = nc.declare_dram_parameter("vocT8", [2 * P, 2 * VSP], F8, isOutput=False)

    o_slab = nc.declare_dram_parameter(
        "o_slab", [P, NT * NH * 1024], BF16, isOutput=True)

    with TileContext(nc) as tc:
        with (
            tc.tile_pool(name="const", bufs=1) as cpool,
            tc.tile_pool(name="apool", bufs=4) as apool,
            tc.tile_pool(name="slabp", bufs=4) as slabp,
            tc.tile_pool(name="psp", bufs=4, space="PSUM") as psp,
        ):
            eTd = [cpool.tile([P, 2 * BS], F8, tag=f"eTd{k}", name=f"eTd{k}")
                   for k in range(2)]
            vocd = [cpool.tile([P, 2 * VSP], F8, tag=f"vocd{k}", name=f"vocd{k}")
                    for k in range(2)]

            # PE warm-up burst while input DMAs are in flight (PE clock ramps
            # to full speed only after ~3us of continuous execution; a gap
            # resets the ramp, so the burst must bridge the load window).
            w0 = cpool.tile([P, 2 * P], F8, tag="w0")
            x0 = cpool.tile([P, 2 * 512], F8, tag="x0")
            nc.vector.memset(w0, 0.0)
            nc.vector.memset(x0, 0.0)
            for i in range(14):
                psw = psp.tile([P, 1024], F32, tag="ps", name=f"ps_warm{i}")
                nc.tensor.matmul(
                    psw[:, 0:512],
                    lhsT=w0.rearrange("p (i m) -> p i m", i=2),
                    rhs=x0.rearrange("p (i v) -> p i v", i=2),
                    start=True, stop=True, perf_mode=DR,
                )

            # input loads spread across the three DMA-capable queues; vocab
            # tables land in two chunks so the first tiles start sooner
            nc.scalar.dma_start(eTd[0], eT8[0:P, :])
            nc.scalar.dma_start(eTd[1], eT8[P:2 * P, :])
            for k in range(2):
                vd = vocd[k].rearrange("p (i v) -> p i v", i=2)
                vs = vocT8[k * P:(k + 1) * P, :].rearrange(
                    "p (i v) -> p i v", i=2)
                eng = nc.sync if k == 0 else nc.gpsimd
                eng.dma_start(vd[:, :, 0:2048], vs[:, :, 0:2048])
                eng.dma_start(vd[:, :, 2048:4096], vs[:, :, 2048:4096])

            for t in range(NT):
                lhs = [eTd[k].rearrange("p (i m) -> p i m", i=2)[:, :, t * P:(t + 1) * P]
                       for k in range(2)]
                # Four 1024-wide spans per tile: A_h = vocab h*2048+[0,1024),
                # B_h = +1024.  PE fills A0, A1 first, then B0, B1 — the
                # scalar copies of the A spans run while the B spans are
                # still streaming, so each DVE pairmax fires the moment its
                # B span completes (no copy->tt chain on the PSUM-release
                # critical path).
                def mm_span(span, base, k2):
                    for c in range(2):
                        off = base + c * 512
                        nc.tensor.matmul(
                            span[:, c * 512:(c + 1) * 512],
                            lhsT=lhs[k2],
                            rhs=vocd[k2].rearrange(
                                "p (i v) -> p i v", i=2)[:, :, off:off + 512],
                            start=(k2 == 0), stop=(k2 == 1),
                            perf_mode=DR, skip_group_check=True,
                        )

                for h in range(NH):
                    sA = psp.tile([P, 1024], F32, tag="ps", name=f"psA{t}_{h}")
                    for k2 in range(2):
                        mm_span(sA, h * 2048, k2)
                    slabA = apool.tile([P, 1024], BF16, tag="slabA",
                                       name=f"slabA{t}_{h}")
                    nc.scalar.copy(slabA, sA[:])
                    sB = psp.tile([P, 1024], F32, tag="ps", name=f"psB{t}_{h}")
                    for k2 in range(2):
                        mm_span(sB, h * 2048 + 1024, k2)
                    col = t * NH + h
                    pm = slabp.tile([P, 1024], BF16, tag="pm")
                    nc.vector.tensor_tensor(
                        out=pm,
                        in0=sB[:],
                        in1=slabA,
                        op=mybir.AluOpType.max,
                    )
                    (nc.gpsimd if col % 2 == 0 else nc.sync).dma_start(
                        o_slab[:, col * 1024:(col + 1) * 1024], pm)

    return nc


def get_nc():
    if "nc" not in _CACHED:
        _CACHED["nc"] = build_nc()
    return _CACHED["nc"]


def _dr_pack(mat):
    """[D, F] -> DoubleRow layout [256, 2*F]: row k2*128+p, col i*F+m holds
    mat[k2*256 + i*128 + p, m]."""
    Dd, F = mat.shape
    assert Dd == 512
    out = np.empty((2 * P, 2 * F), dtype=mat.dtype)
    for k2 in range(2):
        for i in range(2):
            out[k2 * P:(k2 + 1) * P, i * F:(i + 1) * F] = \
                mat[k2 * 256 + i * P: k2 * 256 + i * P + P, :]
    return out


def make_in_maps(preds, emb_weight, target):
    """Host-side input prep: layouts, shards, target-row scaling, fp8 cast."""
    import ml_dtypes
    preds = np.ascontiguousarray(np.asarray(preds, dtype=np.float32))      # [B,D,S]
    emb = np.ascontiguousarray(np.asarray(emb_weight, dtype=np.float32))   # [V,D]
    tgt_idx = np.asarray(target).astype(np.int64).reshape(-1)              # [BS]

    predsN = np.ascontiguousarray(preds.transpose(0, 2, 1).reshape(BS, D))
    nrow = np.sqrt((predsN ** 2).sum(axis=1)).astype(np.float32)
    U = predsN - nrow[:, None] * emb[tgt_idx]                              # [BS,D]
    eT = np.ascontiguousarray(U.T)                                         # [D,BS]
    eT8 = _dr_pack(np.clip(eT, -240.0, 240.0)).astype(ml_dtypes.float8_e4m3)

    vocn = (emb / np.sqrt((emb ** 2).sum(axis=1, keepdims=True))
            ).astype(np.float32)                                           # [V,D]
    vocs = vocn * np.float32(VSCALE)

    in_maps = []
    for c in range(NCORES):
        shard = vocs[c * VS:(c + 1) * VS]
        shardT = np.zeros((D, VSP), dtype=np.float32)
        shardT[:, :VS] = shard.T
        v8 = _dr_pack(np.clip(shardT, -240.0, 240.0)).astype(
            ml_dtypes.float8_e4m3)
        in_maps.append({"eT8": eT8, "vocT8": np.ascontiguousarray(v8)})
    return in_maps


def combine(results, preds, emb_weight, target, pad_id):
    """Host-side unshard: global argmax over the 2:1-reduced device scores,
    exact disambiguation of the pair candidates, exact loss."""
    preds = np.asarray(preds, dtype=np.float32)
    emb = np.asarray(emb_weight, dtype=np.float32)
    tgt_idx = np.asarray(target).astype(np.int64).reshape(-1)

    predsN = preds.transpose(0, 2, 1).reshape(BS, D)
    nrow = np.sqrt((predsN ** 2).sum(axis=1))
    tgtN = emb[tgt_idx]
    U = predsN - nrow[:, None] * tgtN
    vocn = emb / np.sqrt((emb ** 2).sum(axis=1, keepdims=True))

    # global argmax over the 2:1-reduced scores: slab row j = t*128+p,
    # col q = h*1024+o  ->  candidate vocab v = h*2048+o (+1024)
    slab = np.concatenate(
        [np.asarray(r["o_slab"]).reshape(P, NT, NH * 1024).transpose(1, 0, 2)
         .reshape(BS, NH * 1024).astype(np.float32) for r in results],
        axis=1)                                                    # [BS, 8*2048]
    qg = np.argmax(slab, axis=1)
    core = qg >> 11
    q = qg & 2047
    h, o = q >> 10, q & 1023
    v0 = h * 2048 + o                                              # < 4000 always
    v1 = v0 + 1024
    g0 = core * VS + v0
    g1 = core * VS + np.minimum(v1, VS - 1)
    d0 = (U * vocn[g0]).sum(axis=1)
    d1 = np.where(v1 < VS, (U * vocn[g1]).sum(axis=1), -np.inf)
    jglobal = np.where(d1 > d0, g1, g0)

    max_cos = (predsN * vocn[jglobal]).sum(axis=1) / nrow
    s3 = (predsN * tgtN).sum(axis=1)
    s4 = (tgtN * tgtN).sum(axis=1)
    cos_tgt = s3 / (np.sqrt(s4) * nrow)

    diff = np.maximum(np.float32(GAMMA) + max_cos - cos_tgt, 0.0).astype(np.float32)
    mask = tgt_idx != int(np.asarray(pad_id))
    denom = np.float32(mask.sum())
    loss = np.float32(np.where(mask, diff, np.float32(0.0)).sum() / denom)
    return np.asarray(loss, dtype=np.float32)


def run_cores(in_maps, trace=False):
    from concourse.bass_utils import run_bass_kernel_spmd
    nc = get_nc()
    if not nc.is_finalized():
        nc.finalize()
    return run_bass_kernel_spmd(nc, in_maps, list(range(NCORES)), trace=trace)


def kernel(preds, emb_weight, target, pad_id):
    in_maps = make_in_maps(preds, emb_weight, target)
    res = run_cores(in_maps, trace=False)
    return combine(res.results, preds, emb_weight, target, pad_id)


# revision 23
# speedup vs baseline: 1.0104x; 1.0104x over previous
"""MaxMarginLoss Trainium2 kernel (8 NeuronCores, vocab-sharded), v2.

Math (reference):
    out_norm = l2norm(preds^T over D)            [B,S,D]
    voc_norm = l2norm(emb over D)                [V,D]
    tgt      = emb[target]                       [B,S,D]
    d        = out_norm@voc_norm.T - tgt@voc_norm.T
    jmax     = argmax_v d
    loss     = mean_masked(relu(g + cos[jmax] - cos[target]))

Key identity: d = (out_norm - tgt) @ voc_norm.T  -> ONE matmul.  Per-row
positive scaling keeps the argmax, so each device computes
    slab[s,v] = (preds[s] - n_s*tgt[s]) . voc_norm[v]    ( = n_s * d[s,v] )
in fp8(e4m3) with DoubleRow perf mode (double-pumped PE, 2 k-planes per
instruction; ~2x bf16 at the power-throttled ~1.2GHz PE clock).  Each
[128,1024] PSUM span A is copied to SBUF bf16 by the scalar engine while
span B streams; the DVE then emits pairmax = max(B, bf16(A)) — a 2:1
reduction of the slab with the copy cost folded in (only one PSUM operand
per instruction is legal).  The 16 pairmax tiles are DMA'd out; the host
takes the global argmax over the 2:1-reduced scores, disambiguates the two
pair candidates {v, v+1024} with exact fp32 dots, and finishes the loss
exactly (cos at the winner, cos at target, masked mean).  fp8 noise only
affects which near-tie index wins the argmax; its cosine is then computed
exactly, so the loss error stays ~2e-4.
"""

import os
import sys

import numpy as np

for _p in ("/opt/trn_rl_repo", "/root/.axon_site/_ro/trn_rl_repo"):
    if os.path.isdir(_p) and _p not in sys.path:
        sys.path.insert(0, _p)

import concourse.bass as bass
import concourse.bacc as bacc_mod
import concourse.mybir as mybir
from concourse.tile import TileContext

P = 128
B, S, D, V = 4, 512, 512, 32000
BS = B * S                  # 2048 rows
NCORES = 8
VS = V // NCORES            # 4000 vocab rows per core
VSP = 4096                  # padded vocab per core (zeros beyond VS never win)
NT = BS // P                # 16 row tiles
NH = 2                      # vocab halves per tile ([P, 2048] PSUM each)
GAMMA = 0.5
VSCALE = 16.0               # scale on voc_norm so fp8 entries sit in normal range

F32 = mybir.dt.float32
BF16 = mybir.dt.bfloat16
F8 = mybir.dt.float8e4

_CACHED = {}


def build_nc():
    nc = bacc_mod.Bacc()
    DR = mybir.MatmulPerfMode.DoubleRow

    # DoubleRow-packed operands: row r = k2*128 + p holds contraction index
    # k2*256 + i*128 + p in plane i (i stored along the free dim).
    eT8 = nc.declare_dram_parameter("eT8", [2 * P, 2 * BS], F8, isOutput=False)
    vocT8 = nc.declare_dram_parameter("vocT8", [2 * P, 2 * VSP], F8, isOutput=False)

    o_slab = nc.declare_dram_parameter(
        "o_slab", [P, NT * NH * 1024], BF16, isOutput=True)

    with TileContext(nc) as tc:
        with (
            tc.tile_pool(name="const", bufs=1) as cpool,
            tc.tile_pool(name="apool", bufs=4) as apool,
            tc.tile_pool(name="slabp", bufs=4) as slabp,
            tc.tile_pool(name="psp", bufs=4, space="PSUM") as psp,
        ):
            eTd = [cpool.tile([P, 2 * BS], F8, tag=f"eTd{k}", name=f"eTd{k}")
                   for k in range(2)]
            vocd = [cpool.tile([P, 2 * VSP], F8, tag=f"vocd{k}", name=f"vocd{k}")
                    for k in range(2)]

            # PE warm-up burst while input DMAs are in flight (PE clock ramps
            # to full speed only after ~3us of continuous execution; a gap
            # resets the ramp, so the burst must bridge the load window).
            w0 = cpool.tile([P, 2 * P], F8, tag="w0")
            x0 = cpool.tile([P, 2 * 512], F8, tag="x0")
            nc.vector.memset(w0, 0.0)
            nc.vector.memset(x0, 0.0)
            for i in range(14):
                psw = psp.tile([P, 1024], F32, tag="ps", name=f"ps_warm{i}")
                nc.tensor.matmul(
                    psw[:, 0:512],
                    lhsT=w0.rearrange("p (i m) -> p i m", i=2),
                    rhs=x0.rearrange("p (i v) -> p i v", i=2),
                    start=True, stop=True, perf_mode=DR,
                )

            # input loads spread across the three DMA-capable queues; vocab
            # tables land in two chunks so the first tiles start sooner
            nc.scalar.dma_start(eTd[0], eT8[0:P, :])
            nc.scalar.dma_start(eTd[1], eT8[P:2 * P, :])
            for k in range(2):
                vd = vocd[k].rearrange("p (i v) -> p i v", i=2)
                vs = vocT8[k * P:(k + 1) * P, :].rearrange(
                    "p (i v) -> p i v", i=2)
                eng = nc.sync if k == 0 else nc.gpsimd
                eng.dma_start(vd[:, :, 0:2048], vs[:, :, 0:2048])
                eng.dma_start(vd[:, :, 2048:4096], vs[:, :, 2048:4096])

            for t in range(NT):
                lhs = [eTd[k].rearrange("p (i m) -> p i m", i=2)[:, :, t * P:(t + 1) * P]
                       for k in range(2)]
                # Four 1024-wide spans per tile: A_h = vocab h*2048+[0,1024),
                # B_h = +1024.  PE fills A0, A1 first, then B0, B1 — the
                # scalar copies of the A spans run while the B spans are
                # still streaming, so each DVE pairmax fires the moment its
                # B span completes (no copy->tt chain on the PSUM-release
                # critical path).
                def mm_span(span, base, k2):
                    for c in range(2):
                        off = base + c * 512
                        nc.tensor.matmul(
                            span[:, c * 512:(c + 1) * 512],
                            lhsT=lhs[k2],
                            rhs=vocd[k2].rearrange(
                                "p (i v) -> p i v", i=2)[:, :, off:off + 512],
                            start=(k2 == 0), stop=(k2 == 1),
                            perf_mode=DR, skip_group_check=True,
                        )

                for h in range(NH):
                    sA = psp.tile([P, 1024], F32, tag="ps", name=f"psA{t}_{h}")
                    for k2 in range(2):
                        mm_span(sA, h * 2048, k2)
                    slabA = apool.tile([P, 1024], BF16, tag="slabA",
                                       name=f"slabA{t}_{h}")
                    nc.scalar.copy(slabA, sA[:])
                    sB = psp.tile([P, 1024], F32, tag="ps", name=f"psB{t}_{h}")
                    for k2 in range(2):
                        mm_span(sB, h * 2048 + 1024, k2)
                    col = t * NH + h
                    pm = slabp.tile([P, 1024], BF16, tag="pm")
                    nc.vector.tensor_tensor(
                        out=pm,
                        in0=sB[:],
                        in1=slabA,
                        op=mybir.AluOpType.max,
                    )
                    (nc.gpsimd if col % 2 == 0 else nc.sync).dma_start(
                        o_slab[:, col * 1024:(col + 1) * 1024], pm)

    return nc


def get_nc():
    if "nc" not in _CACHED:
        _CACHED["nc"] = build_nc()
    return _CACHED["nc"]


def _dr_pack(mat):
    """[D, F] -> DoubleRow layout [256, 2*F]: row k2*128+p, col i*F+m holds
    mat[k2*256 + i*128 + p, m]."""
    Dd, F = mat.shape
    assert Dd == 512
    out = np.empty((2 * P, 2 * F), dtype=mat.dtype)
    for k2 in range(2):
        for i in range(2):
            out[k2 * P:(k2 + 1) * P, i * F:(i + 1) * F] = \
                mat[k2 * 256 + i * P: k2 * 256 + i * P + P, :]
    return out


def make_in_maps(preds, emb_weight, target):
    """Host-side input prep: layouts, shards, target-row scaling, fp8 cast."""
    import ml_dtypes
    preds = np.ascontiguousarray(np.asarray(preds, dtype=np.float32))      # [B,D,S]
    emb = np.ascontiguousarray(np.asarray(emb_weight, dtype=np.float32))   # [V,D]
    tgt_idx = np.asarray(target).astype(np.int64).reshape(-1)              # [BS]

    predsN = np.ascontiguousarray(preds.transpose(0, 2, 1).reshape(BS, D))
    nrow = np.sqrt((predsN ** 2).sum(axis=1)).astype(np.float32)
    U = predsN - nrow[:, None] * emb[tgt_idx]                              # [BS,D]
    eT = np.ascontiguousarray(U.T)                                         # [D,BS]
    eT8 = _dr_pack(np.clip(eT, -240.0, 240.0)).astype(ml_dtypes.float8_e4m3)

    vocn = (emb / np.sqrt((emb ** 2).sum(axis=1, keepdims=True))
            ).astype(np.float32)                                           # [V,D]
    vocs = vocn * np.float32(VSCALE)

    in_maps = []
    for c in range(NCORES):
        shard = vocs[c * VS:(c + 1) * VS]
        shardT = np.zeros((D, VSP), dtype=np.float32)
        shardT[:, :VS] = shard.T
        v8 = _dr_pack(np.clip(shardT, -240.0, 240.0)).astype(
            ml_dtypes.float8_e4m3)
        in_maps.append({"eT8": eT8, "vocT8": np.ascontiguousarray(v8)})
    return in_maps


def combine(results, preds, emb_weight, target, pad_id):
    """Host-side unshard: global argmax over the 2:1-reduced device scores,
    exact disambiguation of the pair candidates, exact loss."""
    preds = np.asarray(preds, dtype=np.float32)
    emb = np.asarray(emb_weight, dtype=np.float32)
    tgt_idx = np.asarray(target).astype(np.int64).reshape(-1)

    predsN = preds.transpose(0, 2, 1).reshape(BS, D)
    nrow = np.sqrt((predsN ** 2).sum(axis=1))
    tgtN = emb[tgt_idx]
    U = predsN - nrow[:, None] * tgtN
    vocn = emb / np.sqrt((emb ** 2).sum(axis=1, keepdims=True))

    # global argmax over the 2:1-reduced scores: slab row j = t*128+p,
    # col q = h*1024+o  ->  candidate vocab v = h*2048+o (+1024)
    slab = np.concatenate(
        [np.asarray(r["o_slab"]).reshape(P, NT, NH * 1024).transpose(1, 0, 2)
         .reshape(BS, NH * 1024).astype(np.float32) for r in results],
        axis=1)                                                    # [BS, 8*2048]
    qg = np.argmax(slab, axis=1)
    core = qg >> 11
    q = qg & 2047
    h, o = q >> 10, q & 1023
    v0 = h * 2048 + o                                              # < 4000 always
    v1 = v0 + 1024
    g0 = core * VS + v0
    g1 = core * VS + np.minimum(v1, VS - 1)
    d0 = (U * vocn[g0]).sum(axis=1)
    d1 = np.where(v1 < VS, (U * vocn[g1]).sum(axis=1), -np.inf)
    jglobal = np.where(d1 > d0, g1, g0)

    max_cos = (predsN * vocn[jglobal]).sum(axis=1) / nrow
    s3 = (predsN * tgtN).sum(axis=1)
    s4 = (tgtN * tgtN).sum(axis=1)
    cos_tgt = s3 / (np.sqrt(s4) * nrow)

    diff = np.maximum(np.float32(GAMMA) + max_cos - cos_tgt, 0.0).astype(np.float32)
    mask = tgt_idx != int(np.asarray(pad_id))
    denom = np.float32(mask.sum())
    loss = np.float32(np.where(mask, diff, np.float32(0.0)).sum() / denom)
    return np.asarray(loss, dtype=np.float32)


def run_cores(in_maps, trace=False):
    from concourse.bass_utils import run_bass_kernel_spmd
    nc = get_nc()
    if not nc.is_finalized():
        nc.finalize()
    return run_bass_kernel_spmd(nc, in_maps, list(range(NCORES)), trace=trace)


def kernel(preds, emb_weight, target, pad_id):
    in_maps = make_in_maps(preds, emb_weight, target)
    res = run_cores(in_maps, trace=False)
    return combine(res.results, preds, emb_weight, target, pad_id)
